# Initial kernel scaffold
#
"""TRN2 Bass kernel for nn_ClassicalSelfAttention (N=8192, D=1024) on 8 NeuronCores.

Math: out = softmax((X R)(X E)^T / sqrt(D)) X
Rewritten per-core (row-shard m over cores):
    QT = R^T Xi^T              (QT[c,m], via lhsT=R, rhs=XiT)
    PT = E^T QT = (Xi R E^T)^T (PT[d,m], via lhsT=ET, rhs=QT)
    LT_j = XT_j^T PT           ([n_chunk, m] logits, transposed layout)
    softmax over n (partition axis, via accumulate + gpsimd partition reduce)
    out = sum_j E_j^T X_j      (AV, lhsT=exp-scores chunk, rhs=X chunk)

All matmuls run in float32r (fp32 with 11-bit mantissa, full PE rate).
Inputs are pre-rounded to fp32r on the host. PRECISION_SPLIT enables a
3-matmul hi/lo split for the logits chain (near-fp32 accuracy, ~2x PE cost).
"""

import numpy as np

import concourse.bass as bass
import concourse.mybir as mybir
import concourse.tile as tile

N = 8192
D = 1024
NCORES = 8
M = N // NCORES  # rows per core = 1024
G = 512  # m-group rows
NG = M // G  # 2 groups
P = 128
SCALE = 1.0 / 32.0  # 1/sqrt(D)

PRECISION_SPLIT = False  # 3-matmul hi/lo split of the logits chain

F32R = mybir.dt.float32r
F32 = mybir.dt.float32
AX = mybir.AxisListType.X


def _round_fp32r(x: np.ndarray) -> np.ndarray:
    """Round fp32 to fp32r (keep 11 mantissa bits, RNE) like PE expects."""
    u = x.view(np.uint32).astype(np.uint64)
    low = u & 0xFFF
    u = u & ~np.uint64(0xFFF)
    round_up = (low > 0x800) | ((low == 0x800) & ((u >> 12) & 1).astype(bool))
    u = u + np.where(round_up, np.uint64(0x1000), np.uint64(0))
    return u.astype(np.uint32).view(np.float32)


def _split_waits(nc, max_waits: int = 1):
    """walrus here fits only ~1 embedded sync-wait per instruction; hoist
    extras onto standalone NoOps on the same engine immediately before."""
    ctr = 0
    for fn in nc.m.functions:
        for bb in fn.blocks:
            insts = list(bb.instructions)
            out = []
            changed = False
            for inst in insts:
                si = getattr(inst, "sync_info", None)
                waits = list(si.on_wait) if si is not None and si.on_wait else []
                if len(waits) > max_waits:
                    changed = True
                    hoist, keep = waits[:-max_waits], waits[-max_waits:]
                    for i in range(0, len(hoist), max_waits):
                        grp = hoist[i : i + max_waits]
                        nop = mybir.InstNoOp(name=f"I-waitsplit-{ctr}")
                        ctr += 1
                        nop.engine = inst.engine
                        nop.sync_info = mybir.SyncInfo(on_wait=grp, on_update=[])
                        out.append(nop)
                    inst.sync_info = mybir.SyncInfo(
                        on_wait=keep, on_update=list(si.on_update)
                    )
                out.append(inst)
            if changed:
                bb.instructions = out
    return nc


def _build_nc():
    nc = bass.Bass("TRN2", target_bir_lowering=False)
    x_d = nc.dram_tensor("x", [N, D], F32R, kind="ExternalInput").ap()
    xt_d = nc.dram_tensor("xt", [D, N], F32R, kind="ExternalInput").ap()
    r_d = nc.dram_tensor("r", [D, D], F32R, kind="ExternalInput").ap()
    et_d = nc.dram_tensor("et", [D, D], F32R, kind="ExternalInput").ap()
    xit_d = nc.dram_tensor("xit", [D, M], F32R, kind="ExternalInput").ap()
    out_d = nc.dram_tensor("out", [M, D], F32, kind="ExternalOutput").ap()

    KO = D // P  # 8 contraction chunks

    with tile.TileContext(nc) as tc:
        # ---------------- prologue: QT, PT (bounced to DRAM) ----------------
        with (
            tc.tile_pool(name="dram", bufs=1, space="DRAM") as dram_pool,
            tc.tile_pool(name="pro_ps", bufs=4, space="PSUM") as pro_ps,
        ):
            pt_dram = dram_pool.tile([D, M], F32R)
            with tc.tile_pool(name="pro_sb", bufs=1) as pro:
                r_sb = pro.tile([P, KO, D], F32R, name="r_sb")
                et_sb = pro.tile([P, KO, D], F32R, name="et_sb")
                xit_sb = pro.tile([P, KO, M], F32R, name="xit_sb")
                nc.sync.dma_start(r_sb, r_d.rearrange("(ko p) d -> p ko d", p=P))
                nc.sync.dma_start(et_sb, et_d.rearrange("(ko p) d -> p ko d", p=P))
                nc.sync.dma_start(xit_sb, xit_d.rearrange("(ko p) m -> p ko m", p=P))

                qt_sb = pro.tile([P, KO, M], F32R, name="qt_sb")
                # QT[c, m] = sum_d' R[d', c] XiT[d', m]
                for co in range(KO):
                    for mh in range(2):
                        ps = pro_ps.tile([P, 512], F32, name="pro_psum")
                        for k in range(KO):
                            nc.tensor.matmul(
                                ps,
                                r_sb[:, k, co * P : (co + 1) * P],
                                xit_sb[:, k, mh * 512 : (mh + 1) * 512],
                                start=(k == 0),
                                stop=(k == KO - 1),
                            )
                        nc.scalar.copy(qt_sb[:, co, mh * 512 : (mh + 1) * 512], ps)
                pt_sb = pro.tile([P, KO, M], F32R, name="pt_sb")
                # PT[d, m] = sum_c E[d, c]... = sum_c ET[c, d] QT[c, m]
                for do in range(KO):
                    for mh in range(2):
                        ps = pro_ps.tile([P, 512], F32, name="pro_psum")
                        for k in range(KO):
                            nc.tensor.matmul(
                                ps,
                                et_sb[:, k, do * P : (do + 1) * P],
                                qt_sb[:, k, mh * 512 : (mh + 1) * 512],
                                start=(k == 0),
                                stop=(k == KO - 1),
                            )
                        nc.scalar.copy(pt_sb[:, do, mh * 512 : (mh + 1) * 512], ps)
                nc.sync.dma_start(
                    pt_dram.rearrange("(ko p) m -> p ko m", p=P), pt_sb
                )

            # ---------------- main: per m-group ----------------
            NCH = N // 512  # 16 XT stream tiles per group
            with (
                tc.tile_pool(name="lbuf", bufs=1) as lpool,
                tc.tile_pool(name="ptg", bufs=2) as ptg_pool,
                tc.tile_pool(name="xts", bufs=3) as xt_pool,
                tc.tile_pool(name="xs", bufs=3) as x_pool,
                tc.tile_pool(name="es", bufs=3) as e_pool,
                tc.tile_pool(name="outs", bufs=2) as out_pool,
                tc.tile_pool(name="stats", bufs=2) as st_pool,
                tc.tile_pool(name="lt_ps", bufs=2, space="PSUM") as lt_ps,
                tc.tile_pool(name="av_ps", bufs=4, space="PSUM") as av_ps,
            ):
                l_sb = lpool.tile([P, 64, G], F32, name="l_sb")
                for g in range(NG):
                    ptg = ptg_pool.tile([P, KO, G], F32R, name="ptg")
                    nc.sync.dma_start(
                        ptg,
                        pt_dram.rearrange("(ko p) m -> p ko m", p=P)[
                            :, :, g * G : (g + 1) * G
                        ],
                    )
                    rm = st_pool.tile([P, G], F32, name="rm")
                    # --- LT phase over 64 n-chunks of 128 (16 DMA tiles of 512) ---
                    for jt in range(NCH):
                        xt_t = xt_pool.tile([P, KO, 512], F32R, name="xt_t")
                        nc.sync.dma_start(
                            xt_t,
                            xt_d.rearrange("(ko p) n -> p ko n", p=P)[
                                :, :, jt * 512 : (jt + 1) * 512
                            ],
                        )
                        for js in range(4):
                            j = jt * 4 + js
                            ps = lt_ps.tile([P, G], F32, name="lt_psum")
                            for k in range(KO):
                                nc.tensor.matmul(
                                    ps,
                                    xt_t[:, k, js * P : (js + 1) * P],
                                    ptg[:, k],
                                    start=(k == 0),
                                    stop=(k == KO - 1),
                                )
                            nc.scalar.copy(l_sb[:, j], ps)
                            if j == 0:
                                nc.vector.tensor_copy(rm, l_sb[:, 0])
                            else:
                                nc.vector.tensor_tensor(
                                    rm, rm, l_sb[:, j], mybir.AluOpType.max
                                )
                    # --- stats: column max across partitions, broadcast ---
                    maxb = st_pool.tile([P, G], F32, name="maxb")
                    nc.gpsimd.partition_all_reduce(
                        maxb, rm, reduce_op=bass.bass_isa.ReduceOp.max
                    )
                    negmax = st_pool.tile([P, G], F32, name="negmax")
                    nc.vector.tensor_scalar_mul(negmax, maxb, -1.0)
                    # --- exp + AV ---
                    av_acc = [
                        av_ps.tile([P, D], F32, name="av_psum") for _ in range(4)
                    ]
                    sacc = st_pool.tile([P, G], F32, name="sacc")
                    for jt in range(NCH):
                        x_t = x_pool.tile([P, 4, D], F32R, name="x_t")
                        nc.sync.dma_start(
                            x_t,
                            x_d.rearrange("(nt p) d -> p nt d", p=P)[
                                :, jt * 4 : (jt + 1) * 4
                            ],
                        )
                        for js in range(4):
                            j = jt * 4 + js
                            e_t = e_pool.tile([P, G], F32R, name="e_t")
                            tmp = e_pool.tile([P, G], F32, name="tmp")
                            nc.vector.tensor_tensor(
                                tmp, l_sb[:, j], negmax, mybir.AluOpType.add
                            )
                            nc.scalar.activation(
                                e_t, tmp, mybir.ActivationFunctionType.Exp,
                                scale=SCALE,
                            )
                            if j == 0:
                                nc.vector.tensor_copy(sacc, e_t.bitcast(F32))
                            else:
                                nc.vector.tensor_tensor(
                                    sacc, sacc, e_t.bitcast(F32), mybir.AluOpType.add
                                )
                            for mt in range(4):
                                for dh in range(2):
                                    nc.tensor.matmul(
                                        av_acc[mt][:, dh * 512 : (dh + 1) * 512],
                                        e_t[:, mt * P : (mt + 1) * P],
                                        x_t[:, js, dh * 512 : (dh + 1) * 512],
                                        start=(j == 0),
                                        stop=(j == 63),
                                    )
                    # --- sums -> reciprocal [128,1] per m-tile; finalize ---
                    ssum = st_pool.tile([P, G], F32, name="ssum")
                    nc.gpsimd.partition_all_reduce(
                        ssum, sacc, reduce_op=bass.bass_isa.ReduceOp.add
                    )
                    scol = st_pool.tile([P, 4], F32, name="scol")
                    for mt in range(4):
                        nc.sync.dma_start(
                            scol[:, mt : mt + 1],
                            ssum[0:1, mt * P : (mt + 1) * P].rearrange("o p -> p o"),
                        )
                    rcol = st_pool.tile([P, 4], F32, name="rcol")
                    nc.vector.reciprocal(rcol, scol)
                    for mt in range(4):
                        o_sb = out_pool.tile([P, D], F32, name="o_sb")
                        nc.vector.tensor_scalar_mul(
                            o_sb, av_acc[mt], rcol[:, mt : mt + 1]
                        )
                        row0 = g * G + mt * P
                        nc.sync.dma_start(out_d[row0 : row0 + P], o_sb)

    _split_waits(nc)
    return nc


_NC_CACHE = {}


def kernel(**inputs) -> np.ndarray:
    from concourse.bass_utils import run_bass_kernel_spmd

    x = np.asarray(inputs["inputs"], dtype=np.float32)
    rot = np.asarray(inputs["rotation"], dtype=np.float32)
    ent = np.asarray(inputs["entangle"], dtype=np.float32)

    x_r = _round_fp32r(np.ascontiguousarray(x))
    xt_r = _round_fp32r(np.ascontiguousarray(x.T))
    r_r = _round_fp32r(np.ascontiguousarray(rot))
    et_r = _round_fp32r(np.ascontiguousarray(ent.T))

    if "nc" not in _NC_CACHE:
        _NC_CACHE["nc"] = _build_nc()
    nc = _NC_CACHE["nc"]

    in_maps = []
    for c in range(NCORES):
        in_maps.append(
            {
                "x": x_r,
                "xt": xt_r,
                "r": r_r,
                "et": et_r,
                "xit": np.ascontiguousarray(xt_r[:, c * M : (c + 1) * M]),
            }
        )
    res = run_bass_kernel_spmd(nc, in_maps, core_ids=list(range(NCORES)))
    out = np.concatenate([res.results[c]["out"] for c in range(NCORES)], axis=0)
    return out.astype(np.float32)


if __name__ == "__main__":
    rng = np.random.default_rng(0)
    x = rng.standard_normal((N, D)).astype(np.float32)
    r = rng.standard_normal((D, D)).astype(np.float32)
    e = rng.standard_normal((D, D)).astype(np.float32)
    o = kernel(inputs=x, rotation=r, entangle=e)
    print(o.shape, o.dtype)


# revision 24
# speedup vs baseline: 11508.5334x; 11508.5334x over previous
"""TRN2 Bass kernel for nn_ClassicalSelfAttention (N=8192, D=1024) on 8 NeuronCores.

Math: out = softmax((X R)(X E)^T / sqrt(D)) X
Row-sharded over 8 cores (m = rows of the query/output). Per core:
    QT = R^T Xi^T              [c, m]   (lhsT=R, rhs=XiT)
    PT = E^T QT                [d, m]   = (Xi R E^T)^T
    LT_j = XT_j^T PT           [n_j, m] logits, transposed layout
    softmax over n (partition axis; cross-chunk accumulate + gpsimd
    partition_all_reduce; max-shift folded into a DVE add + ACT exp)
    out = sum_j E_j^T X_j      (AV; lhsT=exp-scores chunk, rhs=X chunk)

Precision: the logits chain (QT/PT/LT) uses a 3-matmul fp16 hi/lo split
(hi*hi + hi*lo + lo*hi), giving ~fp32-grade logits at full PE rate.
AV runs in single float32r (fp32 rounded to 11-bit mantissa, full rate).
Inputs are pre-rounded/split on the host.
"""

import numpy as np

import concourse.bass as bass
import concourse.mybir as mybir
import concourse.tile as tile

N = 8192
D = 1024
NCORES = 8
M = N // NCORES  # 1024 rows per core
G = 256  # m-group rows
NG = M // G  # 4 groups
P = 128
KO = D // P  # 8 contraction chunks
NCH = 64  # n-chunks of 128 per group sweep
SCALE = 1.0 / 32.0  # 1/sqrt(D)

F32R = mybir.dt.float32r
F32 = mybir.dt.float32
F16 = mybir.dt.float16


def _round_fp32r(x: np.ndarray) -> np.ndarray:
    """Round fp32 to fp32r (keep 11 mantissa bits, RNE) like the PE expects."""
    u = x.view(np.uint32).astype(np.uint64)
    low = u & np.uint64(0xFFF)
    base = u & ~np.uint64(0xFFF)
    round_up = (low > 0x800) | ((low == 0x800) & (((base >> np.uint64(12)) & np.uint64(1)) == 1))
    out = base + np.where(round_up, np.uint64(0x1000), np.uint64(0))
    return out.astype(np.uint32).view(np.float32)


def _split_f16(x: np.ndarray):
    hi = x.astype(np.float16)
    lo = (x - hi.astype(np.float32)).astype(np.float16)
    return hi, lo


def _split_waits(nc, max_waits: int = 1):
    """walrus in this toolchain fits only ~1 embedded sync-wait per
    instruction; hoist extras onto standalone NoOps on the same engine."""
    ctr = 0
    for fn in nc.m.functions:
        for bb in fn.blocks:
            insts = list(bb.instructions)
            out = []
            changed = False
            for inst in insts:
                si = getattr(inst, "sync_info", None)
                waits = list(si.on_wait) if si is not None and si.on_wait else []
                if len(waits) > max_waits:
                    changed = True
                    hoist, keep = waits[:-max_waits], waits[-max_waits:]
                    for i in range(0, len(hoist), max_waits):
                        nop = mybir.InstNoOp(name=f"I-waitsplit-{ctr}")
                        ctr += 1
                        nop.engine = inst.engine
                        nop.sync_info = mybir.SyncInfo(
                            on_wait=hoist[i : i + max_waits], on_update=[]
                        )
                        out.append(nop)
                    inst.sync_info = mybir.SyncInfo(
                        on_wait=keep, on_update=list(si.on_update)
                    )
                out.append(inst)
            if changed:
                bb.instructions = out
    return nc


def _mm3(nc, ps, lhs_hi, lhs_lo, rhs_hi, rhs_lo, k_range, first, last):
    """Accumulate the 3-product hi/lo split into psum `ps`.
    lhs_*/rhs_* are callables k -> AP. Order groups products sharing the
    stationary operand (lhsT) so weight loads amortize."""
    n = len(k_range)
    for i, k in enumerate(k_range):
        nc.tensor.matmul(
            ps, lhs_hi(k), rhs_hi(k), start=(first and i == 0), stop=False
        )
        nc.tensor.matmul(ps, lhs_hi(k), rhs_lo(k), start=False, stop=False)
        nc.tensor.matmul(
            ps, lhs_lo(k), rhs_hi(k), start=False, stop=(last and i == n - 1)
        )


def build_nc(split_waits: bool = True, reps: int = 1):
    nc = bass.Bass("TRN2", target_bir_lowering=False)
    x_d = nc.dram_tensor("x", [N, D], F16, kind="ExternalInput").ap()
    xth_d = nc.dram_tensor("xth", [D, N], F16, kind="ExternalInput").ap()
    xtl_d = nc.dram_tensor("xtl", [D, N], F16, kind="ExternalInput").ap()
    rh_d = nc.dram_tensor("rh", [D, D], F16, kind="ExternalInput").ap()
    rl_d = nc.dram_tensor("rl", [D, D], F16, kind="ExternalInput").ap()
    eth_d = nc.dram_tensor("eth", [D, D], F16, kind="ExternalInput").ap()
    etl_d = nc.dram_tensor("etl", [D, D], F16, kind="ExternalInput").ap()
    xith_d = nc.dram_tensor("xith", [D, M], F16, kind="ExternalInput").ap()
    xitl_d = nc.dram_tensor("xitl", [D, M], F16, kind="ExternalInput").ap()
    out_d = nc.dram_tensor("out", [M, D], F32, kind="ExternalOutput").ap()

    def r3(ap):  # [D, W] dram -> [128, KO, W]
        return ap.rearrange("(ko p) w -> p ko w", p=P)

    with tile.TileContext(nc) as tc:
        with (
            tc.tile_pool(name="dram", bufs=1, space="DRAM") as dram_pool,
        ):
            pth_dram = dram_pool.tile([D, M], F16, name="pth_dram")
            ptl_dram = dram_pool.tile([D, M], F16, name="ptl_dram")
            nmrow_dram = dram_pool.tile([NG, G], F32, name="nmrow_dram")

            # ---------------- prologue: QT, PT ----------------
            with (
                tc.tile_pool(name="pro", bufs=1) as pro,
                tc.tile_pool(name="pro_ps", bufs=4, space="PSUM") as pro_ps,
            ):
                rh = pro.tile([P, KO, D], F16, name="rh")
                rl = pro.tile([P, KO, D], F16, name="rl")
                eth = pro.tile([P, KO, D], F16, name="eth")
                etl = pro.tile([P, KO, D], F16, name="etl")
                xith = pro.tile([P, KO, M], F16, name="xith")
                xitl = pro.tile([P, KO, M], F16, name="xitl")
                for t, d in (
                    (rh, rh_d), (rl, rl_d), (eth, eth_d), (etl, etl_d),
                    (xith, xith_d), (xitl, xitl_d),
                ):
                    nc.sync.dma_start(t, r3(d))

                qth = pro.tile([P, KO, M], F16, name="qth")
                qtl = pro.tile([P, KO, M], F16, name="qtl")
                # QT[c, m] = sum_d' R[d', c] XiT[d', m]
                for co in range(KO):
                    for mh in range(2):
                        ms = slice(mh * 512, (mh + 1) * 512)
                        ps = pro_ps.tile([P, 512], F32, name="pro_psum")
                        cs = slice(co * P, (co + 1) * P)
                        _mm3(
                            nc, ps,
                            lambda k, cs=cs: rh[:, k, cs],
                            lambda k, cs=cs: rl[:, k, cs],
                            lambda k, ms=ms: xith[:, k, ms],
                            lambda k, ms=ms: xitl[:, k, ms],
                            range(KO), True, True,
                        )
                        nc.scalar.copy(qth[:, co, ms], ps)
                        nc.vector.tensor_tensor(
                            qtl[:, co, ms], ps, qth[:, co, ms],
                            mybir.AluOpType.subtract,
                        )
                pth = pro.tile([P, KO, M], F16, name="pth")
                ptl = pro.tile([P, KO, M], F16, name="ptl")
                # PT[d, m] = sum_c ET[c, d] QT[c, m]
                for do in range(KO):
                    for mh in range(2):
                        ms = slice(mh * 512, (mh + 1) * 512)
                        ps = pro_ps.tile([P, 512], F32, name="pro_psum")
                        ds = slice(do * P, (do + 1) * P)
                        _mm3(
                            nc, ps,
                            lambda k, ds=ds: eth[:, k, ds],
                            lambda k, ds=ds: etl[:, k, ds],
                            lambda k, ms=ms: qth[:, k, ms],
                            lambda k, ms=ms: qtl[:, k, ms],
                            range(KO), True, True,
                        )
                        nc.scalar.copy(pth[:, do, ms], ps)
                        nc.vector.tensor_tensor(
                            ptl[:, do, ms], ps, pth[:, do, ms],
                            mybir.AluOpType.subtract,
                        )
                nc.sync.dma_start(r3(pth_dram[:]), pth)
                nc.sync.dma_start(r3(ptl_dram[:]), ptl)

            # ---------------- main loop over m-groups ----------------
            with (
                tc.tile_pool(name="lbuf", bufs=1) as lpool,
                tc.tile_pool(name="ptg", bufs=2) as ptg_pool,
                tc.tile_pool(name="xts", bufs=2) as xt_pool,
                tc.tile_pool(name="xs", bufs=4) as x_pool,
                tc.tile_pool(name="es", bufs=3) as e_pool,
                tc.tile_pool(name="outs", bufs=2) as out_pool,
                tc.tile_pool(name="stats", bufs=2) as st_pool,
                tc.tile_pool(name="lt_ps", bufs=2, space="PSUM") as lt_ps,
                tc.tile_pool(name="av_ps", bufs=2, space="PSUM") as av_ps,
                tc.tile_pool(name="st_ps", bufs=1, space="PSUM") as st_ps,
            ):
                l_sb = lpool.tile([P, NCH, G], F32, name="l_sb")
                ones1 = lpool.tile([1, P], F32, name="ones1")
                nc.vector.memset(ones1, 1.0)
                ident = lpool.tile([P, P], F32, name="ident")
                from concourse.masks import make_identity

                make_identity(nc, ident)

                def emit_group(g):
                    gs = slice(g * G, (g + 1) * G)
                    ptgh = ptg_pool.tile([P, KO, G], F16, name="ptgh")
                    ptgl = ptg_pool.tile([P, KO, G], F16, name="ptgl")
                    nc.sync.dma_start(ptgh, r3(pth_dram[:])[:, :, gs])
                    nc.sync.dma_start(ptgl, r3(ptl_dram[:])[:, :, gs])
                    rm = st_pool.tile([P, G], F32, name="rm")
                    # --- LT: 16 stream tiles x 4 n-subchunks ---
                    for jt in range(16):
                        ns = slice(jt * 512, (jt + 1) * 512)
                        xth_t = xt_pool.tile([P, KO, 512], F16, name="xth_t")
                        xtl_t = xt_pool.tile([P, KO, 512], F16, name="xtl_t")
                        nc.sync.dma_start(xth_t, r3(xth_d)[:, :, ns])
                        nc.sync.dma_start(xtl_t, r3(xtl_d)[:, :, ns])
                        for js in range(4):
                            j = jt * 4 + js
                            sl = slice(js * P, (js + 1) * P)
                            ps = lt_ps.tile([P, G], F32, name="lt_psum")
                            _mm3(
                                nc, ps,
                                lambda k, sl=sl: xth_t[:, k, sl],
                                lambda k, sl=sl: xtl_t[:, k, sl],
                                lambda k: ptgh[:, k],
                                lambda k: ptgl[:, k],
                                range(KO), True, True,
                            )
                            nc.scalar.copy(l_sb[:, j], ps)
                            if j == 0:
                                nc.vector.tensor_copy(rm, l_sb[:, 0])
                            else:
                                nc.vector.tensor_tensor(
                                    rm, rm, l_sb[:, j], mybir.AluOpType.max
                                )
                    # --- stats: column max over partitions via PE transpose,
                    # negate, DRAM-bounce to a row, broadcast back via K=1 mm
                    nmcol = st_pool.tile([P, 2], F32, name="nmcol")
                    for mt in range(2):
                        t_ps = st_ps.tile([P, P], F32, name="t_ps")
                        nc.tensor.transpose(
                            t_ps, rm[:, mt * P : (mt + 1) * P], ident
                        )
                        nc.vector.tensor_reduce(
                            nmcol[:, mt : mt + 1], t_ps,
                            axis=mybir.AxisListType.X, op=mybir.AluOpType.max,
                            negate=True,
                        )
                    nc.sync.dma_start(
                        nmrow_dram[g].rearrange("(t p) -> p t", p=P), nmcol
                    )
                    negrow = st_pool.tile([1, G], F32, name="negrow")
                    nc.sync.dma_start(
                        negrow, nmrow_dram[g].rearrange("(o w) -> o w", o=1)
                    )
                    nm_ps = st_ps.tile([P, G], F32, name="nm_ps")
                    nc.tensor.matmul(nm_ps, ones1, negrow, start=True, stop=True)
                    negmax = st_pool.tile([P, G], F32, name="negmax")
                    nc.scalar.copy(negmax, nm_ps)
                    # --- exp + AV ---
                    av_acc = [
                        av_ps.tile([P, D], F32, name="av_psum") for _ in range(2)
                    ]
                    sacc = st_pool.tile([P, G], F32, name="sacc")
                    for jt in range(16):
                        for js in range(4):
                            j = jt * 4 + js
                            x_t = x_pool.tile([P, D], F16, name="x_t")
                            nc.sync.dma_start(
                                x_t, x_d[j * P : (j + 1) * P, :]
                            )
                            tmp = e_pool.tile([P, G], F32, name="tmp")
                            e_t = e_pool.tile([P, G], F16, name="e_t")
                            nc.vector.tensor_tensor(
                                tmp, l_sb[:, j], negmax, mybir.AluOpType.add
                            )
                            nc.scalar.activation(
                                e_t, tmp, mybir.ActivationFunctionType.Exp,
                                scale=SCALE,
                            )
                            if j == 0:
                                nc.vector.tensor_copy(sacc, e_t)
                            else:
                                nc.vector.tensor_tensor(
                                    sacc, sacc, e_t, mybir.AluOpType.add
                                )
                            for mt in range(2):
                                for dh in range(2):
                                    nc.tensor.matmul(
                                        av_acc[mt][:, dh * 512 : (dh + 1) * 512],
                                        e_t[:, mt * P : (mt + 1) * P],
                                        x_t[:, dh * 512 : (dh + 1) * 512],
                                        start=(j == 0),
                                        stop=(j == NCH - 1),
                                    )
                    # --- sums -> per-m-tile reciprocal; finalize ---
                    scol = st_pool.tile([P, 2], F32, name="scol")
                    for mt in range(2):
                        t_ps = st_ps.tile([P, P], F32, name="t_ps")
                        nc.tensor.transpose(
                            t_ps, sacc[:, mt * P : (mt + 1) * P], ident
                        )
                        nc.vector.tensor_reduce(
                            scol[:, mt : mt + 1], t_ps,
                            axis=mybir.AxisListType.X, op=mybir.AluOpType.add,
                        )
                    rcol = st_pool.tile([P, 2], F32, name="rcol")
                    nc.vector.reciprocal(rcol, scol)
                    for mt in range(2):
                        o_sb = out_pool.tile([P, D], F32, name="o_sb")
                        nc.vector.tensor_scalar_mul(
                            o_sb, av_acc[mt], rcol[:, mt : mt + 1]
                        )
                        row0 = g * G + mt * P
                        nc.sync.dma_start(out_d[row0 : row0 + P], o_sb)

                if reps == 1:
                    for g in range(NG):
                        emit_group(g)
                else:
                    with tc.For_i(0, reps, 1):
                        for g in range(NG):
                            emit_group(g)

    if split_waits:
        _split_waits(nc)
    return nc


_CACHE = {}


def _prep_inputs(x, rot, ent):
    x_r = np.ascontiguousarray(x).astype(np.float16)
    xt = np.ascontiguousarray(x.T)
    xth, xtl = _split_f16(xt)
    rhh, rll = _split_f16(rot)
    et = np.ascontiguousarray(ent.T)
    eth, etl = _split_f16(et)
    return x_r, xth, xtl, rhh, rll, eth, etl


def kernel(**inputs) -> np.ndarray:
    from concourse.bass_utils import run_bass_kernel_spmd

    x = np.asarray(inputs["inputs"], dtype=np.float32)
    rot = np.asarray(inputs["rotation"], dtype=np.float32)
    ent = np.asarray(inputs["entangle"], dtype=np.float32)

    x_r, xth, xtl, rhh, rll, eth, etl = _prep_inputs(x, rot, ent)

    if "nc" not in _CACHE:
        _CACHE["nc"] = build_nc()
    nc = _CACHE["nc"]

    in_maps = []
    for c in range(NCORES):
        cs = slice(c * M, (c + 1) * M)
        in_maps.append(
            {
                "x": x_r,
                "xth": xth,
                "xtl": xtl,
                "rh": rhh,
                "rl": rll,
                "eth": eth,
                "etl": etl,
                "xith": np.ascontiguousarray(xth[:, cs]),
                "xitl": np.ascontiguousarray(xtl[:, cs]),
            }
        )
    res = run_bass_kernel_spmd(nc, in_maps, core_ids=list(range(NCORES)))
    out = np.concatenate([res.results[c]["out"] for c in range(NCORES)], axis=0)
    return np.ascontiguousarray(out.astype(np.float32))


if __name__ == "__main__":
    rng = np.random.default_rng(0)
    x = rng.standard_normal((N, D)).astype(np.float32)
    r = rng.standard_normal((D, D)).astype(np.float32)
    e = rng.standard_normal((D, D)).astype(np.float32)
    o = kernel(inputs=x, rotation=r, entangle=e)
    print(o.shape, o.dtype, float(np.abs(o).max()))
